# revision 3
# baseline (speedup 1.0000x reference)
"""Trainium2 Bass kernel for nn_CorrProductBlock (equivariant product basis block).

Node-parallel across 8 NeuronCores, ~12800 nodes/core in 25 tiles of 512.

v2 design vs the f32 baseline:
- Host pre-transposes node features into a feature-major fp16 layout packed
  per tile ([128 ch, tile, 4 irrep-slots, 512 nodes]), so the device does no
  PE transposes and HBM traffic is halved (fp16 in / fp16 out).
- Output is produced feature-major ([ch, 4, n] per tile), written fp16, and
  un-transposed on the host.
- Per-element weight gather by one-hot matmul into *paired* PSUM banks so one
  DVE tensor-tensor consumes two gather planes per instruction (PSUM f32
  reads are stuck at 1x; pairing amortizes the fixed overhead).
- Engine balance per tile: PE 18 matmuls; DVE 5 TT ops; ACT evac h + 4 u
  evacs; GPSIMD the |h1|^2 reduction chain + a0a.
"""

import numpy as np

import concourse.bass as bass
import concourse.bacc as bacc
import concourse.mybir as mybir
import concourse.tile as tile
from concourse.bass_utils import run_bass_kernel_spmd

MUL = 128
NUM_ELEM = 64
N_CORES = 8
N_NODES = 100000
TILE_N = 512

F32 = mybir.dt.float32
F16 = mybir.dt.float16

MULT = mybir.AluOpType.mult
ADD = mybir.AluOpType.add


def _bcast_mid(ap, k):
    """[128, T] AP -> [128, k, T] broadcast along a new middle dim."""
    return bass.AP(tensor=ap.tensor, offset=ap.offset,
                   ap=[ap.ap[0], [0, k], ap.ap[-1]])


def _build(ntiles: int, repeat: int = 1):
    """Build the per-core Bass program for `ntiles` tiles of TILE_N nodes.

    repeat>1 wraps the pipeline in a device-side loop (timing amplification
    only — reprocesses the same data).
    """
    nc = bacc.Bacc(num_devices=N_CORES)

    xf = nc.dram_tensor("xf", [128, ntiles * 4 * TILE_N], F16, kind="ExternalInput")
    ohd = nc.dram_tensor("ohd", [NUM_ELEM, ntiles * TILE_N], F16, kind="ExternalInput")
    wpre0 = nc.dram_tensor("wpre0", [MUL, MUL], F16, kind="ExternalInput")
    wpre1 = nc.dram_tensor("wpre1", [MUL, MUL], F16, kind="ExternalInput")
    wco0 = nc.dram_tensor("wco0", [MUL, MUL], F16, kind="ExternalInput")
    wco1 = nc.dram_tensor("wco1", [MUL, MUL], F16, kind="ExternalInput")
    wsc0 = nc.dram_tensor("wsc0", [MUL, MUL], F16, kind="ExternalInput")
    wsc1 = nc.dram_tensor("wsc1", [MUL, MUL], F16, kind="ExternalInput")
    t10 = nc.dram_tensor("t10", [NUM_ELEM, MUL], F16, kind="ExternalInput")
    t11 = nc.dram_tensor("t11", [NUM_ELEM, MUL], F16, kind="ExternalInput")
    t200 = nc.dram_tensor("t200", [NUM_ELEM, MUL], F16, kind="ExternalInput")
    t211 = nc.dram_tensor("t211", [NUM_ELEM, MUL], F16, kind="ExternalInput")
    t201 = nc.dram_tensor("t201", [NUM_ELEM, MUL], F16, kind="ExternalInput")
    y = nc.dram_tensor("y", [128, ntiles * 4 * TILE_N], F16, kind="ExternalOutput")

    with tile.TileContext(nc) as tc:
        with (
            tc.tile_pool(name="singles", bufs=1) as singles,
            tc.tile_pool(name="xin", bufs=3) as xin_pool,
            tc.tile_pool(name="ohp", bufs=3) as oh_pool,
            tc.tile_pool(name="cc", bufs=3) as cc_pool,
            tc.tile_pool(name="sqp", bufs=3) as sq_pool,
            tc.tile_pool(name="ssp", bufs=3) as ss_pool,
            tc.tile_pool(name="tpp", bufs=3) as tp_pool,
            tc.tile_pool(name="tqp", bufs=3) as tq_pool,
            tc.tile_pool(name="zzp", bufs=3) as zz_pool,
            tc.tile_pool(name="a0p", bufs=3) as a0_pool,
            tc.tile_pool(name="a1p", bufs=3) as a1_pool,
            tc.tile_pool(name="outp", bufs=3) as out_pool,
            tc.tile_pool(name="ph", bufs=1, space="PSUM") as ph_pool,
            tc.tile_pool(name="pg", bufs=1, space="PSUM") as pg_pool,
            tc.tile_pool(name="pu", bufs=2, space="PSUM") as pu_pool,
        ):
            def load_w(dram, p, tag):
                t = singles.tile([p, MUL], F16, tag=tag)
                nc.sync.dma_start(out=t, in_=dram[:, :])
                return t

            W_pre0 = load_w(wpre0, 128, "wpre0")
            W_pre1 = load_w(wpre1, 128, "wpre1")
            W_co0 = load_w(wco0, 128, "wco0")
            W_co1 = load_w(wco1, 128, "wco1")
            W_sc0 = load_w(wsc0, 128, "wsc0")
            W_sc1 = load_w(wsc1, 128, "wsc1")
            T_10 = load_w(t10, 64, "t10")
            T_11 = load_w(t11, 64, "t11")
            T_200 = load_w(t200, 64, "t200")
            T_211 = load_w(t211, 64, "t211")
            T_201 = load_w(t201, 64, "t201")

            xf_t = xf.rearrange("p (t q n) -> t p q n", t=ntiles, q=4)
            ohd_t = ohd.rearrange("e (t n) -> t e n", t=ntiles)
            y_t = y.rearrange("p (t q n) -> t p q n", t=ntiles, q=4)

            st = [dict() for _ in range(ntiles)]

            def ok(i):
                return 0 <= i < ntiles

            def stage_load(i):
                if not ok(i):
                    return
                xb = xin_pool.tile([128, 4, TILE_N], F16, tag="xb")
                nc.sync.dma_start(out=xb, in_=xf_t[i])
                st[i]["xb"] = xb

            def stage_oh(i):
                if not ok(i):
                    return
                oh = oh_pool.tile([NUM_ELEM, TILE_N], F16, tag="oh")
                nc.sync.dma_start(out=oh, in_=ohd_t[i])
                st[i]["oh"] = oh

            def stage_pre(i):
                # h[c',q,n] = sum_c Wpre[c,c'] x[c,q,n]
                if not ok(i):
                    return
                xb = st[i]["xb"]
                h = ph_pool.tile([128, 4, TILE_N], F32, tag="h")
                nc.tensor.matmul(h[:, 0, :], W_pre0, xb[:, 0, :], start=True, stop=True)
                for k in range(3):
                    nc.tensor.matmul(h[:, 1 + k, :], W_pre1, xb[:, 1 + k, :],
                                     start=True, stop=True)
                st[i]["h"] = h

            def stage_evac(i):
                # cc = fp16(h); frees the 4 h banks (ACT)
                if not ok(i):
                    return
                ct = cc_pool.tile([128, 4, TILE_N], F16, tag="cc")
                nc.scalar.copy(out=ct, in_=st[i]["h"])
                st[i]["cc"] = ct

            def stage_sq(i):
                # sq = c1*c1 (DVE, 2x)
                if not ok(i):
                    return
                ct = st[i]["cc"]
                sq = sq_pool.tile([128, 3, TILE_N], F16, tag="sq")
                nc.vector.tensor_mul(sq, ct[:, 1:4, :], ct[:, 1:4, :])
                st[i]["sq"] = sq

            def stage_ss(i):
                # ss2 = sq0+sq1+sq2 (GPSIMD)
                if not ok(i):
                    return
                sq = st[i]["sq"]
                ss = ss_pool.tile([128, TILE_N], F16, tag="ss")
                nc.gpsimd.tensor_add(ss, sq[:, 0, :], sq[:, 1, :])
                ss2 = ss_pool.tile([128, TILE_N], F16, tag="ss2")
                nc.gpsimd.tensor_add(ss2, ss, sq[:, 2, :])
                st[i]["ss2"] = ss2

            def stage_gA(i):
                # paired gathers: gA = [g200; g201]
                if not ok(i):
                    return
                oh = st[i]["oh"]
                g = pg_pool.tile([128, 2, TILE_N], F32, tag="g")
                nc.tensor.matmul(g[:, 0, :], T_200, oh, start=True, stop=True)
                nc.tensor.matmul(g[:, 1, :], T_201, oh, start=True, stop=True)
                st[i]["gA"] = g

            def stage_V1(i):
                # [t1; p1] = gA * c0 (DVE, 1x PSUM)
                if not ok(i):
                    return
                ct = st[i]["cc"]
                tp = tp_pool.tile([128, 2, TILE_N], F16, tag="tp")
                nc.vector.tensor_tensor(out=tp, in0=st[i]["gA"],
                                        in1=_bcast_mid(ct[:, 0, :], 2), op=MULT)
                st[i]["tp"] = tp

            def stage_gB(i):
                # gB = [g10; g11] (reuses the pg slot; waits V1)
                if not ok(i):
                    return
                oh = st[i]["oh"]
                g = pg_pool.tile([128, 2, TILE_N], F32, tag="g")
                nc.tensor.matmul(g[:, 0, :], T_10, oh, start=True, stop=True)
                nc.tensor.matmul(g[:, 1, :], T_11, oh, start=True, stop=True)
                st[i]["gB"] = g

            def stage_V2(i):
                # [t2; p2] = gB + [t1; p1] (DVE, 1x PSUM)
                if not ok(i):
                    return
                tq = tq_pool.tile([128, 2, TILE_N], F16, tag="tq")
                nc.vector.tensor_tensor(out=tq, in0=st[i]["gB"], in1=st[i]["tp"],
                                        op=ADD)
                st[i]["tq"] = tq

            def stage_gC(i):
                # gC = [g211] (bank 0 of the pg slot; waits V2)
                if not ok(i):
                    return
                oh = st[i]["oh"]
                g = pg_pool.tile([128, 2, TILE_N], F32, tag="g")
                nc.tensor.matmul(g[:, 0, :], T_211, oh, start=True, stop=True)
                st[i]["gC"] = g

            def stage_V3(i):
                # z = g211 * ss2 (DVE, 1x PSUM)
                if not ok(i):
                    return
                zz = zz_pool.tile([128, TILE_N], F16, tag="zz")
                nc.vector.tensor_tensor(out=zz, in0=st[i]["gC"][:, 0, :],
                                        in1=st[i]["ss2"], op=MULT)
                st[i]["zz"] = zz

            def stage_a0a(i):
                # a0a = c0 * t2 (GPSIMD)
                if not ok(i):
                    return
                ct = st[i]["cc"]
                a0a = a0_pool.tile([128, TILE_N], F16, tag="a0a")
                nc.gpsimd.tensor_tensor(out=a0a, in0=ct[:, 0, :],
                                        in1=st[i]["tq"][:, 0, :], op=MULT)
                st[i]["a0a"] = a0a

            def stage_a1(i):
                # a1 = p2 * c1 (DVE, 2x broadcast)
                if not ok(i):
                    return
                ct = st[i]["cc"]
                a1 = a1_pool.tile([128, 3, TILE_N], F16, tag="a1")
                nc.vector.tensor_tensor(out=a1,
                                        in0=_bcast_mid(st[i]["tq"][:, 1, :], 3),
                                        in1=ct[:, 1:4, :], op=MULT)
                st[i]["a1"] = a1

            def stage_final(i):
                # uT[f,q,n] = Wsc.x + Wco.a ; evac fp16 (ACT); DMA out
                if not ok(i):
                    return
                xb, a1 = st[i]["xb"], st[i]["a1"]
                a0a, zz = st[i]["a0a"], st[i]["zz"]
                out_t = out_pool.tile([128, 4, TILE_N], F16, tag="out")
                for q in range(4):
                    u = pu_pool.tile([128, TILE_N], F32, tag="u")
                    if q == 0:
                        nc.tensor.matmul(u, W_sc0, xb[:, 0, :], start=True, stop=False)
                        nc.tensor.matmul(u, W_co0, a0a, start=False, stop=False)
                        nc.tensor.matmul(u, W_co0, zz, start=False, stop=True)
                    else:
                        nc.tensor.matmul(u, W_sc1, xb[:, q, :], start=True, stop=False)
                        nc.tensor.matmul(u, W_co1, a1[:, q - 1, :], start=False,
                                         stop=True)
                    nc.scalar.copy(out=out_t[:, q, :], in_=u)
                nc.sync.dma_start(out=y_t[i], in_=out_t)
                st[i].clear()

            def body():
                # prologue
                for j in range(min(2, ntiles)):
                    stage_load(j)
                    stage_oh(j)
                stage_pre(0)
                stage_evac(0)
                # main software-pipelined loop
                for i in range(ntiles + 1):
                    stage_load(i + 2)
                    stage_oh(i + 2)
                    stage_pre(i + 1)
                    stage_evac(i + 1)
                    stage_gA(i)
                    stage_V1(i)
                    stage_sq(i)
                    stage_ss(i)
                    stage_final(i - 1)
                    stage_gB(i)
                    stage_V2(i)
                    stage_a0a(i)
                    stage_a1(i)
                    stage_gC(i)
                    stage_V3(i)

            if repeat > 1:
                with tc.For_i(0, repeat, hint_engines=tuple(mybir.ALL_ENGINES)):
                    body()
            else:
                body()

    nc.compile()
    return nc


# ---------------------------------------------------------------- host side

def _prep_weights(inp):
    s = 1.0 / np.sqrt(MUL)
    s3 = 1.0 / np.sqrt(3.0)
    f = lambda a: np.asarray(a, dtype=np.float32)
    h = lambda a: np.ascontiguousarray(a.astype(np.float16))
    w = {}
    w["wpre0"] = h(f(inp["Wpre0"]) * s)
    w["wpre1"] = h(f(inp["Wpre1"]) * s)
    w["wco0"] = h((f(inp["Wprod0"]) @ f(inp["Wout0"])) * (s * s))
    w["wco1"] = h((f(inp["Wprod1"]) @ f(inp["Wout1"])) * (s * s))
    w["wsc0"] = h(f(inp["Wsc0"]) * s)
    w["wsc1"] = h(f(inp["Wsc1"]) * s)
    w["t10"] = h(f(inp["w1_0"]))
    w["t11"] = h(f(inp["w1_1"]))
    w["t200"] = h(f(inp["w2_00"]))
    w["t211"] = h(f(inp["w2_11"]) * s3)
    w["t201"] = h(f(inp["w2_01"]))
    return w


def _make_in_maps(node_feats, node_elems, weights, ntiles):
    """Pack full inputs into per-core feature-major fp16 DRAM images."""
    per_core = ntiles * TILE_N
    n_nodes = node_feats.shape[0]
    per_core_raw = (n_nodes + N_CORES - 1) // N_CORES

    xp = np.zeros((N_CORES, per_core, 512), dtype=np.float32)
    oh = np.zeros((N_CORES, NUM_ELEM, per_core), dtype=np.float16)
    for c in range(N_CORES):
        lo = c * per_core_raw
        hi = min(n_nodes, lo + per_core_raw)
        cnt = max(0, hi - lo)
        if cnt:
            xp[c, :cnt] = node_feats[lo:hi]
            e = np.asarray(node_elems[lo:hi]).astype(np.int64)
            oh[c, e, np.arange(cnt)] = 1.0

    # x0: [C, nt, 512, 128] -> [C, 128, nt, 512]
    x0t = xp[:, :, :MUL].reshape(N_CORES, ntiles, TILE_N, MUL)
    x0t = x0t.transpose(0, 3, 1, 2)
    # x1: [C, nt, 512, 128, 3] -> [C, 128, nt, 3, 512]
    x1t = xp[:, :, MUL:].reshape(N_CORES, ntiles, TILE_N, MUL, 3)
    x1t = x1t.transpose(0, 3, 1, 4, 2)
    xq = np.empty((N_CORES, 128, ntiles, 4, TILE_N), dtype=np.float16)
    xq[:, :, :, 0, :] = x0t
    xq[:, :, :, 1:4, :] = x1t
    xq = np.ascontiguousarray(xq.reshape(N_CORES, 128, ntiles * 4 * TILE_N))
    oh = np.ascontiguousarray(oh)

    return [
        {"xf": xq[c], "ohd": oh[c], **weights} for c in range(N_CORES)
    ]


def _unpack_output(res, n_nodes, ntiles):
    per_core = ntiles * TILE_N
    per_core_raw = (n_nodes + N_CORES - 1) // N_CORES
    out = np.empty((n_nodes, 512), dtype=np.float32)
    for c in range(N_CORES):
        lo = c * per_core_raw
        hi = min(n_nodes, lo + per_core_raw)
        cnt = max(0, hi - lo)
        if not cnt:
            continue
        yc = res.results[c]["y"].reshape(128, ntiles, 4, TILE_N)
        u0 = yc[:, :, 0, :].reshape(128, per_core).T  # [n, 128]
        u1 = yc[:, :, 1:4, :].transpose(1, 3, 0, 2).reshape(per_core, 3 * MUL)
        out[lo:hi, :MUL] = u0[:cnt]
        out[lo:hi, MUL:] = u1[:cnt]
    return out


_cache = {}


def _get_program(ntiles, repeat=1):
    key = (ntiles, repeat)
    if key not in _cache:
        _cache[key] = _build(ntiles, repeat=repeat)
    return _cache[key]


def _run(nc, in_maps, trace=False):
    return run_bass_kernel_spmd(
        nc, in_maps, core_ids=list(range(N_CORES)), trace=trace
    )


def kernel(**inputs):
    inputs = {k: np.asarray(v) for k, v in inputs.items()}
    node_feats = inputs["node_feats"].astype(np.float32, copy=False)
    node_elems = inputs["node_elems"]
    n_nodes = node_feats.shape[0]
    per_core_raw = (n_nodes + N_CORES - 1) // N_CORES
    ntiles = (per_core_raw + TILE_N - 1) // TILE_N

    weights = _prep_weights(inputs)
    in_maps = _make_in_maps(node_feats, node_elems, weights, ntiles)
    nc = _get_program(ntiles)
    res = _run(nc, in_maps)
    return _unpack_output(res, n_nodes, ntiles)
